# revision 10
# baseline (speedup 1.0000x reference)
"""Pairwise squared-Euclidean distance map on 8 TRN2 NeuronCores.

d[b, i, j] = sum_c (a[b, c, i] - b[b, c, j])^2
           = aa[b, i] + bb[b, j] - 2 * <a[b, :, i], b[b, :, j]>

Sharding: data-parallel over the N dimension (rows of the distance map).
Core k computes d[:, k*512:(k+1)*512, :] from a[:, :, k*512:(k+1)*512]
and the full (small) b tensor.

All prep happens ON THE HOST: numpy computes aa/bb and assembles fp8
(e4m3) augmented operands with hi/lo splitting so the fp8 quantization
error cancels to second order:
    cross = (-2a)b = c_hi.b_hi + c_hi.b_lo + c_lo.b_hi (+ dropped 2nd-order)
plus hi/lo norm rows and a ones*128 row: K = 3*64 + 7 + pad = 200
contraction rows, folded [100, 2, *] for the TensorE DoubleRow perf
mode (KI=100 spans all four 32-row PE groups, required for full rate).

Trace-informed structure (ntff profiles across 6 iterations):
- ~7.2 us fixed Tile/runtime preamble before any engine can issue a
  DMA, and ~3 us of fixed semaphore postamble after the last byte.
- The PE clock ramps 0.65->2.4 GHz via HAM after ~3.4 us of DENSE
  matmul activity and re-throttles after ~3.4 us idle, so the schedule
  must be gapless from the first matmul (a single mid-kernel PE gap
  measured ~8 us of damage: re-throttle + cold re-ramp).
- EVERYTHING rides the sync HWDGE FIFO: 7 consolidated loads in strict
  need-order, then the stores. The FIFO itself is the ordering gate —
  the Tile scheduler reorders per-engine program order, and the gpsimd
  SWDGE queue both steals 2-4x the round-robin share (starving the
  mm0-gating loads) and caps at ~160 GB/s (Q7 descriptor-emission
  bound), so it is not used at all.
- Batch 0-1 run CHUNK-MAJOR across their 4 row blocks so rhs chunk q+1
  is first needed ~8 matmul-pairs after chunk q, tolerating the slow
  early DMA (~100-250 GB/s before the chip fully ramps) without gaps.
- 128 DoubleRow matmuls (512 cols each, 2 per 2-bank PSUM tile).
  Warm cadence 216 ns; drains then pace the stream.
- Drains (PSUM->SBUF fp32->fp16, 1 elem/cycle): 1024-wide, measured
  1223 ns on Vector / 1114 ns on Scalar; greedy cumulative balance.
- Store width schedule: 1024 for the first pieces (start the store
  stream as soon as the first drain lands), 2048 mid-early, 4096
  (fully HBM-contiguous rows) for the bulk, and 2x2048 for the last
  block — narrow final stores measured 18-91 GB/s in the throttled
  tail (2 KB/row descriptors + end-of-kernel throttle), costing ~8 us.
- DMA sustains ~420-430 GB/s post-ramp; the ~20.5 MB of HBM traffic on
  one saturated queue is the floor (~49 us) plus head and postamble.
"""

import numpy as np
from contextlib import ExitStack

import concourse.bass as bass
import concourse.bacc as bacc
import concourse.mybir as mybir
from concourse.tile import TileContext
from concourse.bass_utils import run_bass_kernel_spmd

B, C, N, M = 4, 64, 4096, 4096
NCORES = 8
NSH = N // NCORES          # 512 N rows per core
NB = NSH // 128            # 4 row blocks of 128
MC = 512                   # output cols per DoubleRow matmul (1 PSUM bank)
DW = 1024                  # drain width (2 PSUM banks per drain)
KAUG = 200                 # padded contraction rows
KI = KAUG // 2             # folded partition rows for DoubleRow
MCH = 1024                 # rhs chunk width (cols)
NCH = M // MCH             # 4 chunks per batch

F32 = mybir.dt.float32
F16 = mybir.dt.float16
F8 = mybir.dt.float8e4

_CACHE = {}


def _build_nc():
    nc = bacc.Bacc(
        "TRN2",
        target_bir_lowering=False,
        debug=False,
        enable_asserts=True,
        num_devices=NCORES,
    )
    lhs_d = nc.declare_dram_parameter("lhs", [KI, B, 2, NSH], F8, isOutput=False)
    rhs_d = nc.declare_dram_parameter(
        "rhs", [KI, B, NCH, 2, MCH], F8, isOutput=False
    )
    d_d = nc.declare_dram_parameter("d", [B, NSH, M], F16, isOutput=True)

    DR = mybir.MatmulPerfMode.DoubleRow

    with ExitStack() as ctx:
        tc = ctx.enter_context(TileContext(nc))
        lpool = ctx.enter_context(tc.tile_pool(name="lhs", bufs=1))
        rpool = ctx.enter_context(tc.tile_pool(name="rhs", bufs=1))
        stage = ctx.enter_context(tc.tile_pool(name="stage", bufs=16))
        mpsum = ctx.enter_context(tc.tile_pool(name="mpsum", bufs=4, space="PSUM"))

        lts = lpool.tile([KI, B, 2, NSH], F8, tag="lt", name="lt")
        rtc = rpool.tile([KI, B, NCH, 2, MCH], F8, tag="rt", name="rt")

        # All loads on the sync HWDGE FIFO in strict need-order; the
        # FIFO guarantees the mm0 gates move first and keeps the DMA
        # queue deep from the first issue. Stores queue up behind.
        nc.sync.dma_start(out=lts[:, 0], in_=lhs_d[:, 0])
        for ch in range(NCH):
            nc.sync.dma_start(out=rtc[:, 0, ch], in_=rhs_d[:, 0, ch])
        nc.sync.dma_start(out=lts[:, 1:B], in_=lhs_d[:, 1:B])
        for bt in range(1, B):
            nc.sync.dma_start(out=rtc[:, bt], in_=rhs_d[:, bt])

        # Greedy drain balance with measured per-1024-col drain costs.
        bal = [0.0, 0.0]

        def drain(dst, src):
            if bal[0] + 1223 <= bal[1] + 1114:
                bal[0] += 1223
                nc.vector.tensor_copy(dst, src)
            else:
                bal[1] += 1114
                nc.scalar.copy(dst, src)

        def mm_pair(pt, bt, i, q):
            wt = lts[:, bt, :, i * 128 : (i + 1) * 128]
            for h in range(DW // MC):
                so = q * DW + h * MC
                ch, off = so // MCH, so % MCH
                nc.tensor.matmul(
                    pt[:, h * MC : (h + 1) * MC],
                    wt,
                    rtc[:, bt, ch, :, off : off + MC],
                    perf_mode=DR,
                )

        def store(bt, i, c0, c1, st):
            nc.sync.dma_start(
                out=d_d[bt, i * 128 : (i + 1) * 128, c0:c1], in_=st[:, c0:c1]
            )

        # Batches 0-1: chunk-major across the 4 row blocks. Store
        # pieces: batch 0 ships 1024-wide after drains q0/q1 (early
        # stream start) and 2048 after q3; batch 1 ships 2048 after
        # q1/q3.
        for bt in range(2):
            sts = [
                stage.tile([128, M], F16, tag="st", name=f"st{bt}_{i}")
                for i in range(NB)
            ]
            for q in range(M // DW):
                for i in range(NB):
                    pt = mpsum.tile(
                        [128, DW], F32, tag="mp", name=f"mp{bt}_{q}_{i}"
                    )
                    mm_pair(pt, bt, i, q)
                    drain(sts[i][:, q * DW : (q + 1) * DW], pt[:, :])
                    if bt == 0 and q <= 1:
                        store(bt, i, q * DW, (q + 1) * DW, sts[i])
                    elif q % 2 == 1:
                        store(bt, i, (q - 1) * DW, (q + 1) * DW, sts[i])

        # Batches 2-3: block-major, fat fully-contiguous 4096-wide
        # stores (8 KB/row descriptors; narrower stores measured
        # 23-170 GB/s in the throttled end-of-kernel tail).
        for bt in range(2, B):
            for i in range(NB):
                st = stage.tile([128, M], F16, tag="st", name=f"st{bt}_{i}")
                for q in range(M // DW):
                    pt = mpsum.tile(
                        [128, DW], F32, tag="mp", name=f"mp{bt}_{i}_{q}"
                    )
                    mm_pair(pt, bt, i, q)
                    drain(st[:, q * DW : (q + 1) * DW], pt[:, :])
                store(bt, i, 0, M, st)

    nc.compile()
    return nc


def _get_nc():
    if "nc" not in _CACHE:
        _CACHE["nc"] = _build_nc()
    return _CACHE["nc"]


_F8NP = mybir.dt.np(F8)


def _q8(x):
    return np.clip(x, -240.0, 240.0).astype(_F8NP).astype(np.float32)


def _make_in_maps(a, b):
    a = np.asarray(a, dtype=np.float32)
    b = np.asarray(b, dtype=np.float32)
    aa = np.einsum("bcn,bcn->bn", a, a)  # [B, N]
    bb = np.einsum("bcm,bcm->bm", b, b)  # [B, M]

    c = -2.0 * a
    c_hi = _q8(c)
    c_lo = _q8(c - c_hi)
    b_hi = _q8(b)
    b_lo = _q8(b - b_hi)
    A = aa - 64.0
    A_hi = _q8(A)
    A_lo = _q8(A - A_hi)
    Bv = bb - 64.0
    B_hi = _q8(Bv)
    B_lo = _q8(Bv - B_hi)

    lhs = np.zeros([B, KAUG, N], dtype=np.float32)
    rhs = np.zeros([B, KAUG, M], dtype=np.float32)
    lhs[:, 0:64] = c_hi
    rhs[:, 0:64] = b_hi
    lhs[:, 64:128] = c_hi
    rhs[:, 64:128] = b_lo
    lhs[:, 128:192] = c_lo
    rhs[:, 128:192] = b_hi
    lhs[:, 192] = A_hi
    rhs[:, 192] = 1.0
    lhs[:, 193] = A_lo
    rhs[:, 193] = 1.0
    lhs[:, 194] = 1.0
    rhs[:, 194] = B_hi
    lhs[:, 195] = 1.0
    rhs[:, 195] = B_lo
    lhs[:, 196] = 1.0
    rhs[:, 196] = 128.0

    lhs8 = lhs.astype(_F8NP)   # values already on the fp8 grid -> exact
    rhs8 = rhs.astype(_F8NP)
    # fold K rows [200] -> [100, 2] with k = j2*100 + ki (DoubleRow pairing)
    lhs8 = lhs8.reshape(B, 2, KI, N).transpose(2, 0, 1, 3)  # [KI, B, 2, N]
    rhs8 = np.ascontiguousarray(
        rhs8.reshape(B, 2, KI, NCH, MCH).transpose(2, 0, 3, 1, 4)
    )  # [KI, B, NCH, 2, MCH]

    in_maps = []
    for k in range(NCORES):
        lk = lhs8[:, :, :, k * NSH : (k + 1) * NSH]
        in_maps.append(
            {
                "lhs": np.ascontiguousarray(lk),
                "rhs": rhs8,
            }
        )
    return in_maps


def kernel(a, b, _trace=False, _trace_kwargs=None):
    nc = _get_nc()
    in_maps = _make_in_maps(a, b)
    res = run_bass_kernel_spmd(
        nc,
        in_maps,
        core_ids=list(range(NCORES)),
        trace=_trace,
        **(_trace_kwargs or {}),
    )
    out = np.concatenate(
        [res.results[k]["d"] for k in range(NCORES)], axis=1
    ).astype(np.float32)
    if _trace:
        _CACHE["last_results"] = res
    return out


# revision 11
# speedup vs baseline: 1.0700x; 1.0700x over previous
"""Pairwise squared-Euclidean distance map on 8 TRN2 NeuronCores.

d[b, i, j] = sum_c (a[b, c, i] - b[b, c, j])^2
           = aa[b, i] + bb[b, j] - 2 * <a[b, :, i], b[b, :, j]>

Sharding: data-parallel over the N dimension (rows of the distance map).
Core k computes d[:, k*512:(k+1)*512, :] from a[:, :, k*512:(k+1)*512]
and the full (small) b tensor.

All prep happens ON THE HOST: numpy computes aa/bb and assembles fp8
(e4m3) augmented operands with hi/lo splitting so the fp8 quantization
error cancels to second order:
    cross = (-2a)b = c_hi.b_hi + c_hi.b_lo + c_lo.b_hi (+ dropped 2nd-order)
plus hi/lo norm rows and a ones*128 row: K = 3*64 + 7 + pad = 200
contraction rows, folded [100, 2, *] for the TensorE DoubleRow perf
mode (KI=100 spans all four 32-row PE groups, required for full rate).

Trace-informed structure (ntff profiles across 6 iterations):
- ~7.2 us fixed Tile/runtime preamble before any engine can issue a
  DMA, and ~3 us of fixed semaphore postamble after the last byte.
- The PE clock ramps 0.65->2.4 GHz via HAM after ~3.4 us of DENSE
  matmul activity and re-throttles after ~3.4 us idle, so the schedule
  must be gapless from the first matmul (a single mid-kernel PE gap
  measured ~8 us of damage: re-throttle + cold re-ramp).
- EVERYTHING rides the sync HWDGE FIFO: 7 consolidated loads in strict
  need-order, then the stores. The FIFO itself is the ordering gate —
  the Tile scheduler reorders per-engine program order, and the gpsimd
  SWDGE queue both steals 2-4x the round-robin share (starving the
  mm0-gating loads) and caps at ~160 GB/s (Q7 descriptor-emission
  bound), so it is not used at all.
- Batch 0-1 run CHUNK-MAJOR across their 4 row blocks so rhs chunk q+1
  is first needed ~8 matmul-pairs after chunk q, tolerating the slow
  early DMA (~100-250 GB/s before the chip fully ramps) without gaps.
- 128 DoubleRow matmuls (512 cols each, 2 per 2-bank PSUM tile).
  Warm cadence 216 ns; drains then pace the stream.
- Drains (PSUM->SBUF fp32->fp16, 1 elem/cycle): 1024-wide, measured
  1223 ns on Vector / 1114 ns on Scalar; greedy cumulative balance.
- Store width schedule: 1024 for the first pieces (start the store
  stream as soon as the first drain lands), 2048 mid-early, 4096
  (fully HBM-contiguous rows) for the bulk, and 2x2048 for the last
  block — narrow final stores measured 18-91 GB/s in the throttled
  tail (2 KB/row descriptors + end-of-kernel throttle), costing ~8 us.
- DMA sustains ~420-430 GB/s post-ramp; the ~20.5 MB of HBM traffic on
  one saturated queue is the floor (~49 us) plus head and postamble.
"""

import numpy as np
from contextlib import ExitStack

import concourse.bass as bass
import concourse.bacc as bacc
import concourse.mybir as mybir
from concourse.tile import TileContext
from concourse.bass_utils import run_bass_kernel_spmd

B, C, N, M = 4, 64, 4096, 4096
NCORES = 8
NSH = N // NCORES          # 512 N rows per core
NB = NSH // 128            # 4 row blocks of 128
MC = 512                   # output cols per DoubleRow matmul (1 PSUM bank)
DW = 1024                  # drain width (2 PSUM banks per drain)
KAUG = 200                 # padded contraction rows
KI = KAUG // 2             # folded partition rows for DoubleRow
MCH = 1024                 # rhs chunk width (cols)
NCH = M // MCH             # 4 chunks per batch

F32 = mybir.dt.float32
F16 = mybir.dt.float16
F8 = mybir.dt.float8e4

_CACHE = {}


def _build_nc():
    nc = bacc.Bacc(
        "TRN2",
        target_bir_lowering=False,
        debug=False,
        enable_asserts=True,
        num_devices=NCORES,
    )
    lhs_d = nc.declare_dram_parameter("lhs", [KI, B, 2, NSH], F8, isOutput=False)
    rhs_d = nc.declare_dram_parameter(
        "rhs", [KI, B, NCH, 2, MCH], F8, isOutput=False
    )
    d_d = nc.declare_dram_parameter("d", [B, NSH, M], F16, isOutput=True)

    DR = mybir.MatmulPerfMode.DoubleRow

    with ExitStack() as ctx:
        tc = ctx.enter_context(TileContext(nc))
        lpool = ctx.enter_context(tc.tile_pool(name="lhs", bufs=1))
        rpool = ctx.enter_context(tc.tile_pool(name="rhs", bufs=1))
        stage = ctx.enter_context(tc.tile_pool(name="stage", bufs=16))
        mpsum = ctx.enter_context(tc.tile_pool(name="mpsum", bufs=4, space="PSUM"))

        lts = lpool.tile([KI, B, 2, NSH], F8, tag="lt", name="lt")
        rtc = rpool.tile([KI, B, NCH, 2, MCH], F8, tag="rt", name="rt")

        # All loads on the sync HWDGE FIFO in strict need-order; the
        # FIFO guarantees the mm0 gates move first and keeps the DMA
        # queue deep from the first issue. Stores queue up behind.
        nc.sync.dma_start(out=lts[:, 0], in_=lhs_d[:, 0])
        for ch in range(NCH):
            nc.sync.dma_start(out=rtc[:, 0, ch], in_=rhs_d[:, 0, ch])
        nc.sync.dma_start(out=lts[:, 1:B], in_=lhs_d[:, 1:B])
        for bt in range(1, B):
            nc.sync.dma_start(out=rtc[:, bt], in_=rhs_d[:, bt])

        # Greedy drain balance with measured per-1024-col drain costs.
        bal = [0.0, 0.0]

        def drain(dst, src):
            if bal[0] + 1223 <= bal[1] + 1114:
                bal[0] += 1223
                nc.vector.tensor_copy(dst, src)
            else:
                bal[1] += 1114
                nc.scalar.copy(dst, src)

        def mm_pair(pt, bt, i, q):
            wt = lts[:, bt, :, i * 128 : (i + 1) * 128]
            for h in range(DW // MC):
                so = q * DW + h * MC
                ch, off = so // MCH, so % MCH
                nc.tensor.matmul(
                    pt[:, h * MC : (h + 1) * MC],
                    wt,
                    rtc[:, bt, ch, :, off : off + MC],
                    perf_mode=DR,
                )

        def store(bt, i, c0, c1, st):
            nc.sync.dma_start(
                out=d_d[bt, i * 128 : (i + 1) * 128, c0:c1], in_=st[:, c0:c1]
            )

        # Batches 0-1: hybrid order. Chunks 0-1 go chunk-major across
        # the 4 row blocks (so rhs chunk k isn't needed until ~4 pairs
        # after chunk k-1 — slack for the slow early loads), then
        # chunks 2-3 go block-major so each block completes early and
        # ships as ONE fat fully-contiguous 4096-wide store: narrow
        # stores (2-4 KB/row descriptors) measured only ~150-350 GB/s
        # vs ~420-430 for 8 KB/row, and the 1024/2048-wide store mix
        # left a ~6 MB backlog crawling at the throttled tail.
        for bt in range(2):
            sts = [
                stage.tile([128, M], F16, tag="st", name=f"st{bt}_{i}")
                for i in range(NB)
            ]
            for q in range(2):
                for i in range(NB):
                    pt = mpsum.tile(
                        [128, DW], F32, tag="mp", name=f"mp{bt}_{q}_{i}"
                    )
                    mm_pair(pt, bt, i, q)
                    drain(sts[i][:, q * DW : (q + 1) * DW], pt[:, :])
            for i in range(NB):
                for q in range(2, M // DW):
                    pt = mpsum.tile(
                        [128, DW], F32, tag="mp", name=f"mp{bt}_{q}_{i}"
                    )
                    mm_pair(pt, bt, i, q)
                    drain(sts[i][:, q * DW : (q + 1) * DW], pt[:, :])
                store(bt, i, 0, M, sts[i])

        # Batches 2-3: block-major, fat fully-contiguous 4096-wide
        # stores (8 KB/row descriptors; narrower stores measured
        # 23-170 GB/s in the throttled end-of-kernel tail).
        for bt in range(2, B):
            for i in range(NB):
                st = stage.tile([128, M], F16, tag="st", name=f"st{bt}_{i}")
                for q in range(M // DW):
                    pt = mpsum.tile(
                        [128, DW], F32, tag="mp", name=f"mp{bt}_{i}_{q}"
                    )
                    mm_pair(pt, bt, i, q)
                    drain(st[:, q * DW : (q + 1) * DW], pt[:, :])
                store(bt, i, 0, M, st)

    nc.compile()
    return nc


def _get_nc():
    if "nc" not in _CACHE:
        _CACHE["nc"] = _build_nc()
    return _CACHE["nc"]


_F8NP = mybir.dt.np(F8)


def _q8(x):
    return np.clip(x, -240.0, 240.0).astype(_F8NP).astype(np.float32)


def _make_in_maps(a, b):
    a = np.asarray(a, dtype=np.float32)
    b = np.asarray(b, dtype=np.float32)
    aa = np.einsum("bcn,bcn->bn", a, a)  # [B, N]
    bb = np.einsum("bcm,bcm->bm", b, b)  # [B, M]

    c = -2.0 * a
    c_hi = _q8(c)
    c_lo = _q8(c - c_hi)
    b_hi = _q8(b)
    b_lo = _q8(b - b_hi)
    A = aa - 64.0
    A_hi = _q8(A)
    A_lo = _q8(A - A_hi)
    Bv = bb - 64.0
    B_hi = _q8(Bv)
    B_lo = _q8(Bv - B_hi)

    lhs = np.zeros([B, KAUG, N], dtype=np.float32)
    rhs = np.zeros([B, KAUG, M], dtype=np.float32)
    lhs[:, 0:64] = c_hi
    rhs[:, 0:64] = b_hi
    lhs[:, 64:128] = c_hi
    rhs[:, 64:128] = b_lo
    lhs[:, 128:192] = c_lo
    rhs[:, 128:192] = b_hi
    lhs[:, 192] = A_hi
    rhs[:, 192] = 1.0
    lhs[:, 193] = A_lo
    rhs[:, 193] = 1.0
    lhs[:, 194] = 1.0
    rhs[:, 194] = B_hi
    lhs[:, 195] = 1.0
    rhs[:, 195] = B_lo
    lhs[:, 196] = 1.0
    rhs[:, 196] = 128.0

    lhs8 = lhs.astype(_F8NP)   # values already on the fp8 grid -> exact
    rhs8 = rhs.astype(_F8NP)
    # fold K rows [200] -> [100, 2] with k = j2*100 + ki (DoubleRow pairing)
    lhs8 = lhs8.reshape(B, 2, KI, N).transpose(2, 0, 1, 3)  # [KI, B, 2, N]
    rhs8 = np.ascontiguousarray(
        rhs8.reshape(B, 2, KI, NCH, MCH).transpose(2, 0, 3, 1, 4)
    )  # [KI, B, NCH, 2, MCH]

    in_maps = []
    for k in range(NCORES):
        lk = lhs8[:, :, :, k * NSH : (k + 1) * NSH]
        in_maps.append(
            {
                "lhs": np.ascontiguousarray(lk),
                "rhs": rhs8,
            }
        )
    return in_maps


def kernel(a, b, _trace=False, _trace_kwargs=None):
    nc = _get_nc()
    in_maps = _make_in_maps(a, b)
    res = run_bass_kernel_spmd(
        nc,
        in_maps,
        core_ids=list(range(NCORES)),
        trace=_trace,
        **(_trace_kwargs or {}),
    )
    out = np.concatenate(
        [res.results[k]["d"] for k in range(NCORES)], axis=1
    ).astype(np.float32)
    if _trace:
        _CACHE["last_results"] = res
    return out


# revision 13
# speedup vs baseline: 1.0726x; 1.0024x over previous
"""Pairwise squared-Euclidean distance map on 8 TRN2 NeuronCores.

d[b, i, j] = sum_c (a[b, c, i] - b[b, c, j])^2
           = aa[b, i] + bb[b, j] - 2 * <a[b, :, i], b[b, :, j]>

Sharding: data-parallel over the N dimension (rows of the distance map).
Core k computes d[:, k*512:(k+1)*512, :] from a[:, :, k*512:(k+1)*512]
and the full (small) b tensor.

All prep happens ON THE HOST: numpy computes aa/bb and assembles fp8
(e4m3) augmented operands with hi/lo splitting so the fp8 quantization
error cancels to second order:
    cross = (-2a)b = c_hi.b_hi + c_hi.b_lo + c_lo.b_hi (+ dropped 2nd-order)
plus hi/lo norm rows and a ones*128 row: K = 3*64 + 7 + pad = 200
contraction rows, folded [100, 2, *] for the TensorE DoubleRow perf
mode (KI=100 spans all four 32-row PE groups, required for full rate).

Trace-informed structure (ntff profiles across 6 iterations):
- ~7.2 us fixed Tile/runtime preamble before any engine can issue a
  DMA, and ~3 us of fixed semaphore postamble after the last byte.
- The PE clock ramps 0.65->2.4 GHz via HAM after ~3.4 us of DENSE
  matmul activity and re-throttles after ~3.4 us idle, so the schedule
  must be gapless from the first matmul (a single mid-kernel PE gap
  measured ~8 us of damage: re-throttle + cold re-ramp).
- EVERYTHING rides the sync HWDGE FIFO: 7 consolidated loads in strict
  need-order, then the stores. The FIFO itself is the ordering gate —
  the Tile scheduler reorders per-engine program order, and the gpsimd
  SWDGE queue both steals 2-4x the round-robin share (starving the
  mm0-gating loads) and caps at ~160 GB/s (Q7 descriptor-emission
  bound), so it is not used at all.
- Batch 0-1 run CHUNK-MAJOR across their 4 row blocks so rhs chunk q+1
  is first needed ~8 matmul-pairs after chunk q, tolerating the slow
  early DMA (~100-250 GB/s before the chip fully ramps) without gaps.
- 128 DoubleRow matmuls (512 cols each, 2 per 2-bank PSUM tile).
  Warm cadence 216 ns; drains then pace the stream.
- Drains (PSUM->SBUF fp32->fp16, 1 elem/cycle): 1024-wide, measured
  1223 ns on Vector / 1114 ns on Scalar; greedy cumulative balance.
- Store width schedule: 1024 for the first pieces (start the store
  stream as soon as the first drain lands), 2048 mid-early, 4096
  (fully HBM-contiguous rows) for the bulk, and 2x2048 for the last
  block — narrow final stores measured 18-91 GB/s in the throttled
  tail (2 KB/row descriptors + end-of-kernel throttle), costing ~8 us.
- DMA sustains ~420-430 GB/s post-ramp; the ~20.5 MB of HBM traffic on
  one saturated queue is the floor (~49 us) plus head and postamble.
"""

import numpy as np
from contextlib import ExitStack

import concourse.bass as bass
import concourse.bacc as bacc
import concourse.mybir as mybir
from concourse.tile import TileContext
from concourse.bass_utils import run_bass_kernel_spmd

B, C, N, M = 4, 64, 4096, 4096
NCORES = 8
NSH = N // NCORES          # 512 N rows per core
NB = NSH // 128            # 4 row blocks of 128
MC = 512                   # output cols per DoubleRow matmul (1 PSUM bank)
DW = 1024                  # drain width (2 PSUM banks per drain)
KAUG = 200                 # padded contraction rows
KI = KAUG // 2             # folded partition rows for DoubleRow
MCH = 1024                 # rhs chunk width (cols)
NCH = M // MCH             # 4 chunks per batch

F32 = mybir.dt.float32
F16 = mybir.dt.float16
F8 = mybir.dt.float8e4

_CACHE = {}


def _build_nc():
    nc = bacc.Bacc(
        "TRN2",
        target_bir_lowering=False,
        debug=False,
        enable_asserts=True,
        num_devices=NCORES,
    )
    lhs_d = nc.declare_dram_parameter("lhs", [KI, B, 2, NSH], F8, isOutput=False)
    rhs_d = nc.declare_dram_parameter(
        "rhs", [KI, B, NCH, 2, MCH], F8, isOutput=False
    )
    d_d = nc.declare_dram_parameter("d", [B, NSH, M], F16, isOutput=True)

    DR = mybir.MatmulPerfMode.DoubleRow

    with ExitStack() as ctx:
        tc = ctx.enter_context(TileContext(nc))
        lpool = ctx.enter_context(tc.tile_pool(name="lhs", bufs=1))
        rpool = ctx.enter_context(tc.tile_pool(name="rhs", bufs=1))
        stage = ctx.enter_context(tc.tile_pool(name="stage", bufs=16))
        mpsum = ctx.enter_context(tc.tile_pool(name="mpsum", bufs=4, space="PSUM"))

        lts = lpool.tile([KI, B, 2, NSH], F8, tag="lt", name="lt")
        rtc = rpool.tile([KI, B, NCH, 2, MCH], F8, tag="rt", name="rt")

        # All loads on the sync HWDGE FIFO in strict need-order; the
        # FIFO guarantees the mm0 gates move first and keeps the DMA
        # queue deep from the first issue. Stores queue up behind.
        nc.sync.dma_start(out=lts[:, 0], in_=lhs_d[:, 0])
        for ch in range(NCH):
            nc.sync.dma_start(out=rtc[:, 0, ch], in_=rhs_d[:, 0, ch])
        nc.sync.dma_start(out=lts[:, 1:B], in_=lhs_d[:, 1:B])
        for bt in range(1, B):
            nc.sync.dma_start(out=rtc[:, bt], in_=rhs_d[:, bt])

        # Greedy drain balance with measured per-1024-col drain costs.
        bal = [0.0, 0.0]

        def drain(dst, src):
            if bal[0] + 1223 <= bal[1] + 1114:
                bal[0] += 1223
                nc.vector.tensor_copy(dst, src)
            else:
                bal[1] += 1114
                nc.scalar.copy(dst, src)

        def mm_pair(pt, bt, i, q):
            wt = lts[:, bt, :, i * 128 : (i + 1) * 128]
            for h in range(DW // MC):
                so = q * DW + h * MC
                ch, off = so // MCH, so % MCH
                nc.tensor.matmul(
                    pt[:, h * MC : (h + 1) * MC],
                    wt,
                    rtc[:, bt, ch, :, off : off + MC],
                    perf_mode=DR,
                )

        def store(bt, i, c0, c1, st):
            nc.sync.dma_start(
                out=d_d[bt, i * 128 : (i + 1) * 128, c0:c1], in_=st[:, c0:c1]
            )

        # Batches 0-1: hybrid order. Chunks 0-1 go chunk-major across
        # the 4 row blocks (so rhs chunk k isn't needed until ~4 pairs
        # after chunk k-1 — slack for the slow early loads), then
        # chunks 2-3 go block-major so each block completes early and
        # ships as ONE fat fully-contiguous 4096-wide store: narrow
        # stores (2-4 KB/row descriptors) measured only ~150-350 GB/s
        # vs ~420-430 for 8 KB/row, and the 1024/2048-wide store mix
        # left a ~6 MB backlog crawling at the throttled tail.
        for bt in range(2):
            sts = [
                stage.tile([128, M], F16, tag="st", name=f"st{bt}_{i}")
                for i in range(NB)
            ]
            for q in range(2):
                for i in range(NB):
                    pt = mpsum.tile(
                        [128, DW], F32, tag="mp", name=f"mp{bt}_{q}_{i}"
                    )
                    mm_pair(pt, bt, i, q)
                    drain(sts[i][:, q * DW : (q + 1) * DW], pt[:, :])
                    if bt == 0 and q == 1:
                        # ship the ready halves early: fills the
                        # load->store transition dip (~20-25 us)
                        store(bt, i, 0, 2 * DW, sts[i])
            for i in range(NB):
                for q in range(2, M // DW):
                    pt = mpsum.tile(
                        [128, DW], F32, tag="mp", name=f"mp{bt}_{q}_{i}"
                    )
                    mm_pair(pt, bt, i, q)
                    drain(sts[i][:, q * DW : (q + 1) * DW], pt[:, :])
                if bt == 0:
                    store(bt, i, 2 * DW, M, sts[i])
                else:
                    store(bt, i, 0, M, sts[i])

        # Batches 2-3: block-major, fat fully-contiguous 4096-wide
        # stores (8 KB/row descriptors; narrower stores measured
        # 23-170 GB/s in the throttled end-of-kernel tail).
        for bt in range(2, B):
            for i in range(NB):
                st = stage.tile([128, M], F16, tag="st", name=f"st{bt}_{i}")
                for q in range(M // DW):
                    pt = mpsum.tile(
                        [128, DW], F32, tag="mp", name=f"mp{bt}_{i}_{q}"
                    )
                    mm_pair(pt, bt, i, q)
                    drain(st[:, q * DW : (q + 1) * DW], pt[:, :])
                store(bt, i, 0, M, st)

        # Tail warmers: dead-write dummy matmuls keep the PE busy while
        # the store backlog ships. The chip re-throttles to 50% ~4.5 us
        # after the PE goes idle (measured: tail DMA drops 423 -> ~280
        # GB/s), stretching the last ~2.7 MB by ~5 us. These consume no
        # DMA and end before the last store byte.
        for w in range(26):
            pt = mpsum.tile([128, DW], F32, tag="mp", name=f"tail{w}")
            nc.tensor.matmul(
                pt[:, 0:MC],
                lts[:, B - 1, :, NSH - 128 : NSH],
                rtc[:, B - 1, NCH - 1, :, 0:MC],
                perf_mode=DR,
            )

    nc.compile()
    return nc


def _get_nc():
    if "nc" not in _CACHE:
        _CACHE["nc"] = _build_nc()
    return _CACHE["nc"]


_F8NP = mybir.dt.np(F8)


def _q8(x):
    return np.clip(x, -240.0, 240.0).astype(_F8NP).astype(np.float32)


def _make_in_maps(a, b):
    a = np.asarray(a, dtype=np.float32)
    b = np.asarray(b, dtype=np.float32)
    aa = np.einsum("bcn,bcn->bn", a, a)  # [B, N]
    bb = np.einsum("bcm,bcm->bm", b, b)  # [B, M]

    c = -2.0 * a
    c_hi = _q8(c)
    c_lo = _q8(c - c_hi)
    b_hi = _q8(b)
    b_lo = _q8(b - b_hi)
    A = aa - 64.0
    A_hi = _q8(A)
    A_lo = _q8(A - A_hi)
    Bv = bb - 64.0
    B_hi = _q8(Bv)
    B_lo = _q8(Bv - B_hi)

    lhs = np.zeros([B, KAUG, N], dtype=np.float32)
    rhs = np.zeros([B, KAUG, M], dtype=np.float32)
    lhs[:, 0:64] = c_hi
    rhs[:, 0:64] = b_hi
    lhs[:, 64:128] = c_hi
    rhs[:, 64:128] = b_lo
    lhs[:, 128:192] = c_lo
    rhs[:, 128:192] = b_hi
    lhs[:, 192] = A_hi
    rhs[:, 192] = 1.0
    lhs[:, 193] = A_lo
    rhs[:, 193] = 1.0
    lhs[:, 194] = 1.0
    rhs[:, 194] = B_hi
    lhs[:, 195] = 1.0
    rhs[:, 195] = B_lo
    lhs[:, 196] = 1.0
    rhs[:, 196] = 128.0

    lhs8 = lhs.astype(_F8NP)   # values already on the fp8 grid -> exact
    rhs8 = rhs.astype(_F8NP)
    # fold K rows [200] -> [100, 2] with k = j2*100 + ki (DoubleRow pairing)
    lhs8 = lhs8.reshape(B, 2, KI, N).transpose(2, 0, 1, 3)  # [KI, B, 2, N]
    rhs8 = np.ascontiguousarray(
        rhs8.reshape(B, 2, KI, NCH, MCH).transpose(2, 0, 3, 1, 4)
    )  # [KI, B, NCH, 2, MCH]

    in_maps = []
    for k in range(NCORES):
        lk = lhs8[:, :, :, k * NSH : (k + 1) * NSH]
        in_maps.append(
            {
                "lhs": np.ascontiguousarray(lk),
                "rhs": rhs8,
            }
        )
    return in_maps


def kernel(a, b, _trace=False, _trace_kwargs=None):
    nc = _get_nc()
    in_maps = _make_in_maps(a, b)
    res = run_bass_kernel_spmd(
        nc,
        in_maps,
        core_ids=list(range(NCORES)),
        trace=_trace,
        **(_trace_kwargs or {}),
    )
    out = np.concatenate(
        [res.results[k]["d"] for k in range(NCORES)], axis=1
    ).astype(np.float32)
    if _trace:
        _CACHE["last_results"] = res
    return out
